# revision 12
# baseline (speedup 1.0000x reference)
"""Chamfer distance loss on 8 Trainium2 NeuronCores.

Problem: template/source [4, 4096, 3] f32 -> scalar loss
  d[b,n,m] = ||t_n - s_m||^2 ; mean_n(min_m d) + mean_m(min_n d), mean over b.

Strategy (data-parallel over batch x template-half, 2 cores per batch):
  Each core handles one batch's full source set (4096 pts) against one half
  of the template set (2048 pts). Distances come from a single K=5 matmul in
  NEGATED form: with augmented vectors
     srcA  = [sx, sy, sz, ||s||^2, 1]         (stationary, lhsT)
     tmplA = [2tx, 2ty, 2tz, -1, -||t||^2]    (moving, rhs)
  dot = 2 s.t - ||s||^2 - ||t||^2 = -d. Working with -d lets every reduction
  be a MAX (DVE tensor ops + gpsimd partition_all_reduce support max, not min).

  Per source block i (32 blocks of 128):
    PE:  4 matmuls -> PSUM [128 src, 2048 tmpl] fp32 (= -d tile)
    ACT: cast PSUM -> SBUF fp16 (drains PSUM; fp16 costs ~1.7e-5 rel err)
    DVE: tensor_tensor_reduce(max) folds the tile in half and row-reduces it
         -> negd10 column i (per-src max over tmpl = -min d)
         tensor_tensor(max) accumulates colwise running max -> acc [128, 2048]
  Tail: gpsimd partition_all_reduce(max) over acc partitions -> negd01 row.
  Host: negate, combine halves, means in float64.
"""

import numpy as np

import concourse.bass as bass
import concourse.bass_isa as bass_isa
import concourse.tile as tile
from concourse import mybir
from concourse.bass_utils import run_bass_kernel_spmd
from concourse.vector_clock import ScopedClock

B, N, M = 4, 4096, 4096
HALF = N // 2  # template half per core: 2048
N_CORES = 8
SRC_BLOCKS = M // 128  # 32
FREE = HALF  # 2048 free-dim columns per core
NEG_INF = -3.0e38

F32 = mybir.dt.float32
F16 = mybir.dt.float16

_MAX_DRAIN_WAITS = 1


class _ChunkedDrainTileContext(tile.TileContext):
    """The walrus build used by the axon/PJRT path rejects instructions with
    more than a few sync waits; Tile's exit drain attaches one wait per live
    logical processor. Split them across sequential drains (same semantics)."""

    def _drain_and_barrier(self, tick_clock, wait_clock):
        drain_inst = self.nc.sync.drain()
        wait_clock.add_sem_waits(
            drain_inst.ins, ScopedClock({None: tick_clock.global_clock})
        )
        si = drain_inst.ins.sync_info
        if si is not None and si.on_wait and len(si.on_wait) > _MAX_DRAIN_WAITS:
            waits = list(si.on_wait)
            drain_inst.ins.sync_info = mybir.SyncInfo(
                on_wait=waits[:_MAX_DRAIN_WAITS], on_update=list(si.on_update or [])
            )
            rest = waits[_MAX_DRAIN_WAITS:]
            while rest:
                d = self.nc.sync.drain()
                d.ins.sync_info = mybir.SyncInfo(
                    on_wait=rest[:_MAX_DRAIN_WAITS], on_update=[]
                )
                rest = rest[_MAX_DRAIN_WAITS:]

        self.nc.all_engine_barrier()
        assert self.sems is not None
        popped = self.nc._tile_sem_poison_stack.pop()
        assert popped is self._sem_poison
        self.nc.clear_and_free_semaphores(list(self.sems.allocated().values()))
        self.nc.all_engine_barrier()


def _split_multi_waits(nc: bass.Bass, max_waits: int = 1) -> int:
    """This walrus build rejects instructions carrying more than a couple of
    sync waits. Hoist excess waits onto NoOp instructions inserted just before
    the offender on the same engine — same-engine program order preserves the
    semantics (all waits still complete before the instruction issues)."""
    n = 0
    for f in nc.m.functions:
        for bb in f.blocks:
            insts = list(bb.instructions)
            out = []
            changed = False
            for inst in insts:
                si = inst.sync_info
                if si is not None and si.on_wait and len(si.on_wait) > max_waits:
                    waits = list(si.on_wait)
                    extra, keep = waits[:-max_waits], waits[-max_waits:]
                    while extra:
                        chunk, extra = extra[:max_waits], extra[max_waits:]
                        n += 1
                        out.append(
                            mybir.InstNoOp(
                                name=f"waitsplit-{n}",
                                engine=inst.engine,
                                sync_info=mybir.SyncInfo(on_wait=chunk, on_update=[]),
                            )
                        )
                    inst.sync_info = mybir.SyncInfo(
                        on_wait=keep, on_update=list(si.on_update or [])
                    )
                    changed = True
                out.append(inst)
            if changed:
                bb.instructions = out
    return n


def build_program() -> bass.Bass:
    nc = bass.Bass("TRN2", target_bir_lowering=True, debug=False)
    srcA = nc.declare_dram_parameter("srcA", [5, M], F32, isOutput=False)
    tmplA = nc.declare_dram_parameter("tmplA", [5, FREE], F32, isOutput=False)
    ident = nc.declare_dram_parameter("ident", [128, 128], F16, isOutput=False)
    negd10 = nc.declare_dram_parameter("negd10", [128, SRC_BLOCKS], F32, isOutput=True)
    # negd01[n_loc, t] corresponds to template point t*128 + n_loc
    negd01 = nc.declare_dram_parameter("negd01", [128, FREE // 128], F32, isOutput=True)

    with _ChunkedDrainTileContext(nc) as tc:
        with (
            tc.tile_pool(name="inp", bufs=1) as inp,
            tc.tile_pool(name="psum", bufs=2, space="PSUM") as pp,
            tc.tile_pool(name="cast", bufs=4) as castp,
            tc.tile_pool(name="accp", bufs=2) as accp,
            tc.tile_pool(name="scr", bufs=2) as scrp,
            tc.tile_pool(name="outp", bufs=1) as outp,
        ):
            src_sb = inp.tile([5, M], F32)
            nc.sync.dma_start(src_sb[:], srcA[:])
            tmpl_sb = inp.tile([5, FREE], F32)
            nc.sync.dma_start(tmpl_sb[:], tmplA[:])
            id_sb = inp.tile([128, 128], F16)
            nc.sync.dma_start(id_sb[:], ident[:])

            d10sb = outp.tile([128, SRC_BLOCKS], F32)
            acc = None
            for i in range(SRC_BLOCKS):
                ps = pp.tile([128, FREE], F32)
                for j in range(FREE // 512):
                    nc.tensor.matmul(
                        ps[:, bass.ts(j, 512)],
                        lhsT=src_sb[:, bass.ts(i, 128)],
                        rhs=tmpl_sb[:, bass.ts(j, 512)],
                        start=True,
                        stop=True,
                    )
                if i == 0:
                    ctile = accp.tile([128, FREE], F16, tag="acc")
                else:
                    ctile = castp.tile([128, FREE], F16, tag="cast")
                nc.scalar.copy(ctile[:], ps[:])

                # row-max of the tile: halving folds (t_t fp16 SBUF runs 2x)
                # then a short 1x reduce.
                s1 = scrp.tile([128, FREE // 2], F16, tag="s1")
                nc.vector.tensor_tensor(
                    s1[:],
                    ctile[:, 0 : FREE // 2],
                    ctile[:, FREE // 2 : FREE],
                    op=mybir.AluOpType.max,
                )
                s2 = scrp.tile([128, FREE // 4], F16, tag="s2")
                nc.vector.tensor_tensor(
                    s2[:],
                    s1[:, 0 : FREE // 4],
                    s1[:, FREE // 4 : FREE // 2],
                    op=mybir.AluOpType.max,
                )
                nc.vector.tensor_reduce(
                    d10sb[:, i : i + 1],
                    s2[:],
                    axis=mybir.AxisListType.X,
                    op=mybir.AluOpType.max,
                )
                if i == 0:
                    acc = ctile
                else:
                    acc_new = accp.tile([128, FREE], F16, tag="acc")
                    nc.vector.tensor_tensor(
                        acc_new[:], acc[:], ctile[:], op=mybir.AluOpType.max
                    )
                    acc = acc_new

            # partition-axis max: PE-transpose acc (16 blocks of 128x128) into
            # PSUM, then a single DVE X-reduce per 128-column group.
            psT = pp.tile([128, FREE], F16, tag="ps")
            for t in range(FREE // 128):
                nc.tensor.transpose(
                    psT[:, bass.ts(t, 128)], acc[:, bass.ts(t, 128)], id_sb[:]
                )
            d01t = outp.tile([128, FREE // 128], F32)
            nc.vector.tensor_reduce(
                d01t[:],
                psT[:].rearrange("p (t c) -> p t c", c=128),
                axis=mybir.AxisListType.X,
                op=mybir.AluOpType.max,
            )
            nc.sync.dma_start(negd01[:], d01t[:])
            nc.sync.dma_start(negd10[:], d10sb[:])
    _split_multi_waits(nc)
    return nc


_PROGRAM = None


def get_program() -> bass.Bass:
    global _PROGRAM
    if _PROGRAM is None:
        _PROGRAM = build_program()
    return _PROGRAM


def make_in_maps(template: np.ndarray, source: np.ndarray) -> list[dict]:
    """Host-side prep: augmented 5-dim representations, sharded per core.
    Core c -> batch c//2, template half c%2."""
    template = np.asarray(template, dtype=np.float32)
    source = np.asarray(source, dtype=np.float32)
    in_maps = []
    for c in range(N_CORES):
        b, h = divmod(c, 2)
        s = source[b]  # [M, 3]
        t = template[b, h * HALF : (h + 1) * HALF]  # [HALF, 3]
        sn = (s * s).sum(-1)
        tn = (t * t).sum(-1)
        srcA = np.stack(
            [s[:, 0], s[:, 1], s[:, 2], sn, np.ones_like(sn)], 0
        ).astype(np.float32)
        tmplA = np.stack(
            [2 * t[:, 0], 2 * t[:, 1], 2 * t[:, 2], -np.ones_like(tn), -tn], 0
        ).astype(np.float32)
        in_maps.append(
            {
                "srcA": np.ascontiguousarray(srcA),
                "tmplA": np.ascontiguousarray(tmplA),
                "ident": np.eye(128, dtype=np.float16),
            }
        )
    return in_maps


def combine(results: list[dict]) -> np.ndarray:
    """Gather per-core partials into the scalar loss (float64 accumulation)."""
    per_batch = []
    for b in range(B):
        r0, r1 = results[2 * b], results[2 * b + 1]
        # negd10[p, i] = max over this core's template half of -d(src_{i*128+p}, .)
        nd10 = np.maximum(r0["negd10"], r1["negd10"]).astype(np.float64)
        d10_mean = -nd10.mean()
        # negd01[n_loc, t] -> template index t*128 + n_loc within the half
        nd01 = np.concatenate(
            [r0["negd01"].T.reshape(-1), r1["negd01"].T.reshape(-1)]
        ).astype(np.float64)
        d01_mean = -nd01.mean()
        per_batch.append(d01_mean + d10_mean)
    return np.asarray(np.mean(per_batch), dtype=np.float32)


def kernel(template: np.ndarray, source: np.ndarray) -> np.ndarray:
    nc = get_program()
    in_maps = make_in_maps(template, source)
    res = run_bass_kernel_spmd(nc, in_maps, list(range(N_CORES)))
    return combine(res.results)


# revision 14
# speedup vs baseline: 2.1760x; 2.1760x over previous
"""Chamfer distance loss on 8 Trainium2 NeuronCores.

Problem: template/source [4, 4096, 3] f32 -> scalar loss
  d[b,n,m] = ||t_n - s_m||^2 ; mean_n(min_m d) + mean_m(min_n d), mean over b.

Strategy (data-parallel over batch x template-half, 2 cores per batch):
  Each core handles one batch's full source set (4096 pts) against one half of
  the template set (2048 pts). Distances come from a single matmul in NEGATED
  split-bf16 form: every fp32 operand is decomposed into 3 bf16 terms
  (x = x1+x2+x3, each the bf16 rounding of the residual) and all cross
  products with combined magnitude above ~2^-27 are carried as separate K
  rows, so one K=24 bf16 matmul reproduces
      negd = 2 t.s - ||t||^2 - ||s||^2  = -d
  to near-fp32 accuracy at full PE rate (fp32 matmul is ~4x slower and
  disables fast weight load). Working with -d makes every reduction a MAX.

  Per (template block i of 16, source half h of 2):
    PE:  2 matmuls [24,1024] -> PSUM [128 tmpl, 2048 src] fp32 (= -d tile)
    ACT: cast PSUM -> SBUF fp16 (drains PSUM; fp16 adds ~1.7e-5 rel err)
    DVE: fold chain (tensor_tensor max, fp16 SBUF = 2x rate) + short reduce
         -> negd01 column (per-template max over the src half)
         tensor_tensor(max) running acc per half -> acc_h [128, 2048]
  Tail: PE-transpose acc_h via identity matmul, DVE X-reduce -> negd10.
  Host: negate, combine halves, means in float64.
"""

import numpy as np
import ml_dtypes

import concourse.bass as bass
import concourse.tile as tile
from concourse import mybir
from concourse.bass_utils import run_bass_kernel_spmd
from concourse.vector_clock import ScopedClock

B, N, M = 4, 4096, 4096
HALF = N // 2  # template half per core: 2048
N_CORES = 8
TBLOCKS = HALF // 128  # 16 template blocks
SFREE = M // 2  # source half width: 2048
K = 24

F32 = mybir.dt.float32
F16 = mybir.dt.float16
BF16 = mybir.dt.bfloat16

_MAX_DRAIN_WAITS = 1


class _ChunkedDrainTileContext(tile.TileContext):
    """The walrus build used by the axon/PJRT path rejects instructions with
    more than a couple of sync waits; Tile's exit drain attaches one wait per
    live logical processor. Split them across sequential drains."""

    def _drain_and_barrier(self, tick_clock, wait_clock):
        drain_inst = self.nc.sync.drain()
        wait_clock.add_sem_waits(
            drain_inst.ins, ScopedClock({None: tick_clock.global_clock})
        )
        si = drain_inst.ins.sync_info
        if si is not None and si.on_wait and len(si.on_wait) > _MAX_DRAIN_WAITS:
            waits = list(si.on_wait)
            drain_inst.ins.sync_info = mybir.SyncInfo(
                on_wait=waits[:_MAX_DRAIN_WAITS], on_update=list(si.on_update or [])
            )
            rest = waits[_MAX_DRAIN_WAITS:]
            while rest:
                d = self.nc.sync.drain()
                d.ins.sync_info = mybir.SyncInfo(
                    on_wait=rest[:_MAX_DRAIN_WAITS], on_update=[]
                )
                rest = rest[_MAX_DRAIN_WAITS:]

        self.nc.all_engine_barrier()
        assert self.sems is not None
        popped = self.nc._tile_sem_poison_stack.pop()
        assert popped is self._sem_poison
        self.nc.clear_and_free_semaphores(list(self.sems.allocated().values()))
        self.nc.all_engine_barrier()


def _split_multi_waits(nc: bass.Bass, max_waits: int = 1) -> int:
    """This walrus build rejects instructions carrying several sync waits.
    Hoist excess waits onto NoOps inserted before the offender on the same
    engine — same-engine program order preserves the semantics."""
    n = 0
    for f in nc.m.functions:
        for bb in f.blocks:
            insts = list(bb.instructions)
            out = []
            changed = False
            for inst in insts:
                si = inst.sync_info
                if si is not None and si.on_wait and len(si.on_wait) > max_waits:
                    waits = list(si.on_wait)
                    extra, keep = waits[:-max_waits], waits[-max_waits:]
                    while extra:
                        chunk, extra = extra[:max_waits], extra[max_waits:]
                        n += 1
                        out.append(
                            mybir.InstNoOp(
                                name=f"waitsplit-{n}",
                                engine=inst.engine,
                                sync_info=mybir.SyncInfo(on_wait=chunk, on_update=[]),
                            )
                        )
                    inst.sync_info = mybir.SyncInfo(
                        on_wait=keep, on_update=list(si.on_update or [])
                    )
                    changed = True
                out.append(inst)
            if changed:
                bb.instructions = out
    return n


def build_program() -> bass.Bass:
    nc = bass.Bass("TRN2", target_bir_lowering=True, debug=False)
    tmplA = nc.declare_dram_parameter("tmplA", [K, HALF], BF16, isOutput=False)
    srcA = nc.declare_dram_parameter("srcA", [K, M], BF16, isOutput=False)
    ident = nc.declare_dram_parameter("ident", [128, 128], F16, isOutput=False)
    # negd01[p, i*2+h] = max over src half h of -d(tmpl_{i*128+p}, .)
    negd01 = nc.declare_dram_parameter("negd01", [128, 2 * TBLOCKS], F32, isOutput=True)
    # negd10[n_loc, h*16+t] corresponds to source point h*2048 + t*128 + n_loc
    negd10 = nc.declare_dram_parameter("negd10", [128, M // 128], F32, isOutput=True)

    with _ChunkedDrainTileContext(nc) as tc:
        with (
            tc.tile_pool(name="inp", bufs=1) as inp,
            tc.tile_pool(name="psum", bufs=2, space="PSUM") as pp,
            tc.tile_pool(name="cast", bufs=4) as castp,
            tc.tile_pool(name="acc0p", bufs=2) as acc0p,
            tc.tile_pool(name="acc1p", bufs=2) as acc1p,
            tc.tile_pool(name="scr", bufs=2) as scrp,
            tc.tile_pool(name="outp", bufs=1) as outp,
        ):
            tmpl_sb = inp.tile([K, HALF], BF16)
            nc.sync.dma_start(tmpl_sb[:], tmplA[:])
            src_sb = inp.tile([K, M], BF16)
            nc.sync.dma_start(src_sb[:], srcA[:])
            id_sb = inp.tile([128, 128], F16)
            nc.sync.dma_start(id_sb[:], ident[:])

            d01sb = outp.tile([128, 2 * TBLOCKS], F32)
            accp = [acc0p, acc1p]
            accs = [None, None]
            for i in range(TBLOCKS):
                for h in range(2):
                    ps = pp.tile([128, SFREE], F32, tag="ps")
                    for jj in range(SFREE // 512):
                        nc.tensor.matmul(
                            ps[:, bass.ts(jj, 512)],
                            lhsT=tmpl_sb[:, bass.ts(i, 128)],
                            rhs=src_sb[:, h * SFREE + jj * 512 : h * SFREE + (jj + 1) * 512],
                            start=True,
                            stop=True,
                        )
                    if i == 0:
                        ctile = accp[h].tile([128, SFREE], F16, tag=f"acc{h}")
                    else:
                        ctile = castp.tile([128, SFREE], F16, tag="cast")
                    nc.scalar.copy(ctile[:], ps[:])

                    # per-template row max over this src half: fold chain + reduce
                    s1 = scrp.tile([128, SFREE // 2], F16, tag="s1")
                    nc.vector.tensor_tensor(
                        s1[:],
                        ctile[:, 0 : SFREE // 2],
                        ctile[:, SFREE // 2 : SFREE],
                        op=mybir.AluOpType.max,
                    )
                    s2 = scrp.tile([128, SFREE // 4], F16, tag="s2")
                    nc.vector.tensor_tensor(
                        s2[:],
                        s1[:, 0 : SFREE // 4],
                        s1[:, SFREE // 4 : SFREE // 2],
                        op=mybir.AluOpType.max,
                    )
                    nc.vector.tensor_reduce(
                        d01sb[:, 2 * i + h : 2 * i + h + 1],
                        s2[:],
                        axis=mybir.AxisListType.X,
                        op=mybir.AluOpType.max,
                    )
                    if i > 0:
                        acc_new = accp[h].tile([128, SFREE], F16, tag=f"acc{h}")
                        nc.vector.tensor_tensor(
                            acc_new[:], accs[h][:], ctile[:], op=mybir.AluOpType.max
                        )
                        accs[h] = acc_new
                    else:
                        accs[h] = ctile

            # partition-axis max of each acc: PE-transpose 128x128 blocks into
            # PSUM then one DVE X-reduce per acc.
            d10t = outp.tile([128, M // 128], F32)
            for h in range(2):
                psT = pp.tile([128, SFREE], F16, tag="ps")
                for t in range(SFREE // 128):
                    nc.tensor.transpose(
                        psT[:, bass.ts(t, 128)], accs[h][:, bass.ts(t, 128)], id_sb[:]
                    )
                nc.vector.tensor_reduce(
                    d10t[:, h * 16 : (h + 1) * 16],
                    psT[:].rearrange("p (t c) -> p t c", c=128),
                    axis=mybir.AxisListType.X,
                    op=mybir.AluOpType.max,
                )
            nc.sync.dma_start(negd10[:], d10t[:])
            nc.sync.dma_start(negd01[:], d01sb[:])
    _split_multi_waits(nc)
    return nc


_PROGRAM = None


def get_program() -> bass.Bass:
    global _PROGRAM
    if _PROGRAM is None:
        _PROGRAM = build_program()
    return _PROGRAM


def _split3(x: np.ndarray):
    bf = ml_dtypes.bfloat16
    h1 = x.astype(bf).astype(np.float32)
    h2 = (x - h1).astype(bf).astype(np.float32)
    h3 = (x - h1 - h2).astype(bf).astype(np.float32)
    return h1, h2, h3


# cross-product levels kept: everything with combined magnitude >= ~2^-27
_PAIRS = [(0, 0), (0, 1), (1, 0), (0, 2), (1, 1), (2, 0)]


def make_in_maps(template: np.ndarray, source: np.ndarray) -> list[dict]:
    """Host-side prep: split-bf16 augmented K=24 representations, sharded per
    core. Core c -> batch c//2, template half c%2."""
    template = np.asarray(template, dtype=np.float32)
    source = np.asarray(source, dtype=np.float32)
    bf = ml_dtypes.bfloat16
    in_maps = []
    for c in range(N_CORES):
        b, hh = divmod(c, 2)
        t = template[b, hh * HALF : (hh + 1) * HALF]  # [HALF, 3]
        s = source[b]  # [M, 3]
        T = _split3(t)
        U = _split3((2.0 * s).astype(np.float32))
        nt = (t.astype(np.float64) ** 2).sum(-1).astype(np.float32)
        ns = (s.astype(np.float64) ** 2).sum(-1).astype(np.float32)
        NT = _split3(nt)
        NS = _split3(ns)
        ones_t = np.ones_like(nt)
        ones_s = np.ones_like(ns)
        a_rows, b_rows = [], []
        for cc in range(3):
            for (ii, jj) in _PAIRS:
                a_rows.append(T[ii][:, cc])
                b_rows.append(U[jj][:, cc])
        for kk in range(3):
            a_rows.append(-NT[kk])
            b_rows.append(ones_s)
            a_rows.append(-ones_t)
            b_rows.append(NS[kk])
        tmplA = np.stack(a_rows, 0).astype(bf)  # [K, HALF]
        srcA = np.stack(b_rows, 0).astype(bf)  # [K, M]
        in_maps.append(
            {
                "tmplA": np.ascontiguousarray(tmplA),
                "srcA": np.ascontiguousarray(srcA),
                "ident": np.eye(128, dtype=np.float16),
            }
        )
    return in_maps


def combine(results: list[dict]) -> np.ndarray:
    """Gather per-core partials into the scalar loss (float64 accumulation)."""
    per_batch = []
    for b in range(B):
        r0, r1 = results[2 * b], results[2 * b + 1]
        d01_parts = []
        for r in (r0, r1):
            nd = r["negd01"].astype(np.float64)  # [128, 32]
            # columns 2i, 2i+1 are the two src halves of template block i
            nd01 = np.maximum(nd[:, 0::2], nd[:, 1::2])  # [128, 16]
            # template index within half = i*128 + p -> transpose to [16,128]
            d01_parts.append(nd01.T.reshape(-1))
        d01 = -np.concatenate(d01_parts)  # [4096]
        # negd10[n_loc, h*16+t] for source index h*2048 + t*128 + n_loc
        nd10 = np.maximum(
            r0["negd10"].astype(np.float64), r1["negd10"].astype(np.float64)
        )
        d10 = -nd10.T.reshape(-1)  # [32,128] -> index t'*128+n_loc with t'=h*16+t
        per_batch.append(d01.mean() + d10.mean())
    return np.asarray(np.mean(per_batch), dtype=np.float32)


def kernel(template: np.ndarray, source: np.ndarray) -> np.ndarray:
    nc = get_program()
    in_maps = make_in_maps(template, source)
    res = run_bass_kernel_spmd(nc, in_maps, list(range(N_CORES)))
    return combine(res.results)


# revision 20
# speedup vs baseline: 2.2331x; 1.0263x over previous
"""Chamfer distance loss on 8 Trainium2 NeuronCores.

Problem: template/source [4, 4096, 3] f32 -> scalar loss
  d[b,n,m] = ||t_n - s_m||^2 ; mean_n(min_m d) + mean_m(min_n d), mean over b.

Strategy (data-parallel over batch x template-half, 2 cores per batch):
  Each core handles one batch's full source set (4096 pts) against one half of
  the template set (2048 pts). Distances come from a single matmul in NEGATED
  split-bf16 form: every fp32 operand is decomposed into 3 bf16 terms
  (x = x1+x2+x3, each the bf16 rounding of the residual) and all cross
  products with combined magnitude above ~2^-27 are carried as separate K
  rows, so one K=24 bf16 matmul reproduces
      negd = 2 t.s - ||t||^2 - ||s||^2  = -d
  to near-fp32 accuracy at full PE rate (fp32 matmul is ~4x slower and
  disables fast weight load). Working with -d makes every reduction a MAX.

  Per (template block i of 16, source half h of 2):
    PE:  2 matmuls [24,1024] -> PSUM [128 tmpl, 2048 src] fp32 (= -d tile)
    ACT: cast PSUM -> SBUF fp16 (drains PSUM; fp16 adds ~1.7e-5 rel err)
    DVE: fold chain (tensor_tensor max, fp16 SBUF = 2x rate) + short reduce
         -> negd01 column (per-template max over the src half)
         tensor_tensor(max) running acc per half -> acc_h [128, 2048]
  Tail: PE-transpose acc_h via identity matmul, DVE X-reduce -> negd10.
  Host: negate, combine halves, means in float64.
"""

import numpy as np
import ml_dtypes

import concourse.bass as bass
import concourse.tile as tile
from concourse import mybir
from concourse.bass_utils import run_bass_kernel_spmd
from concourse.vector_clock import ScopedClock

B, N, M = 4, 4096, 4096
HALF = N // 2  # template half per core: 2048
N_CORES = 8
TBLOCKS = HALF // 128  # 16 template blocks
SFREE = M // 2  # source half width: 2048
K = 24

F32 = mybir.dt.float32
F16 = mybir.dt.float16
BF16 = mybir.dt.bfloat16

_MAX_DRAIN_WAITS = 1


class _ChunkedDrainTileContext(tile.TileContext):
    """The walrus build used by the axon/PJRT path rejects instructions with
    more than a couple of sync waits; Tile's exit drain attaches one wait per
    live logical processor. Split them across sequential drains."""

    def _drain_and_barrier(self, tick_clock, wait_clock):
        drain_inst = self.nc.sync.drain()
        wait_clock.add_sem_waits(
            drain_inst.ins, ScopedClock({None: tick_clock.global_clock})
        )
        si = drain_inst.ins.sync_info
        if si is not None and si.on_wait and len(si.on_wait) > _MAX_DRAIN_WAITS:
            waits = list(si.on_wait)
            drain_inst.ins.sync_info = mybir.SyncInfo(
                on_wait=waits[:_MAX_DRAIN_WAITS], on_update=list(si.on_update or [])
            )
            rest = waits[_MAX_DRAIN_WAITS:]
            while rest:
                d = self.nc.sync.drain()
                d.ins.sync_info = mybir.SyncInfo(
                    on_wait=rest[:_MAX_DRAIN_WAITS], on_update=[]
                )
                rest = rest[_MAX_DRAIN_WAITS:]

        self.nc.all_engine_barrier()
        assert self.sems is not None
        popped = self.nc._tile_sem_poison_stack.pop()
        assert popped is self._sem_poison
        self.nc.clear_and_free_semaphores(list(self.sems.allocated().values()))
        self.nc.all_engine_barrier()


def _split_multi_waits(nc: bass.Bass, max_waits: int = 1) -> int:
    """This walrus build rejects instructions carrying several sync waits.
    Hoist excess waits onto NoOps inserted before the offender on the same
    engine — same-engine program order preserves the semantics."""
    n = 0
    for f in nc.m.functions:
        for bb in f.blocks:
            insts = list(bb.instructions)
            out = []
            changed = False
            for inst in insts:
                si = inst.sync_info
                if si is not None and si.on_wait and len(si.on_wait) > max_waits:
                    waits = list(si.on_wait)
                    extra, keep = waits[:-max_waits], waits[-max_waits:]
                    while extra:
                        chunk, extra = extra[:max_waits], extra[max_waits:]
                        n += 1
                        out.append(
                            mybir.InstNoOp(
                                name=f"waitsplit-{n}",
                                engine=inst.engine,
                                sync_info=mybir.SyncInfo(on_wait=chunk, on_update=[]),
                            )
                        )
                    inst.sync_info = mybir.SyncInfo(
                        on_wait=keep, on_update=list(si.on_update or [])
                    )
                    changed = True
                out.append(inst)
            if changed:
                bb.instructions = out
    return n


def build_program() -> bass.Bass:
    nc = bass.Bass("TRN2", target_bir_lowering=True, debug=False)
    tmplA = nc.declare_dram_parameter("tmplA", [K, HALF], BF16, isOutput=False)
    srcA = nc.declare_dram_parameter("srcA", [K, M], BF16, isOutput=False)
    ident = nc.declare_dram_parameter("ident", [128, 128], F16, isOutput=False)
    # negd01[p, i] = max over all src of -d(tmpl_{i*128+p}, .)
    negd01 = nc.declare_dram_parameter("negd01", [128, TBLOCKS], F32, isOutput=True)
    # negd10[n_loc, t] corresponds to source point t*128 + n_loc
    negd10 = nc.declare_dram_parameter("negd10", [128, M // 128], F32, isOutput=True)

    with _ChunkedDrainTileContext(nc) as tc:
        with (
            tc.tile_pool(name="inp", bufs=1) as inp,
            tc.tile_pool(name="psum", bufs=2, space="PSUM") as pp,
            tc.tile_pool(name="cast", bufs=5) as castp,
            tc.tile_pool(name="acc0p", bufs=2) as acc0p,
            tc.tile_pool(name="acc1p", bufs=2) as acc1p,
            tc.tile_pool(name="scr", bufs=2) as scrp,
            tc.tile_pool(name="outp", bufs=1) as outp,
        ):
            tmpl_sb = inp.tile([K, HALF], BF16)
            nc.sync.dma_start(tmpl_sb[:], tmplA[:])
            src_sb = inp.tile([K, M], BF16)
            nc.sync.dma_start(src_sb[:], srcA[:])
            id_sb = inp.tile([128, 128], F16)
            nc.sync.dma_start(id_sb[:], ident[:])

            d01sb = outp.tile([128, TBLOCKS], F32)
            # Running max over template blocks, split into two independent
            # chains: even blocks on DVE, odd blocks on GPSIMD (otherwise
            # idle), merged once at the end.
            acc_dve = None
            acc_gp = None
            ctiles = []
            for i in range(TBLOCKS):
                # one [128, M] cast tile per template block; 2 psum tiles
                ctile = castp.tile([128, M], F16, tag="cast")
                for h in range(2):
                    ps = pp.tile([128, SFREE], F32, tag="ps")
                    for jj in range(SFREE // 512):
                        nc.tensor.matmul(
                            ps[:, bass.ts(jj, 512)],
                            lhsT=tmpl_sb[:, bass.ts(i, 128)],
                            rhs=src_sb[:, h * SFREE + jj * 512 : h * SFREE + (jj + 1) * 512],
                            start=True,
                            stop=True,
                        )
                    nc.scalar.copy(ctile[:, h * SFREE : (h + 1) * SFREE], ps[:])

                # per-template row max over all 4096 src: fold chain + reduce
                s1 = scrp.tile([128, M // 2], F16, tag="s1")
                nc.vector.tensor_tensor(
                    s1[:], ctile[:, 0 : M // 2], ctile[:, M // 2 : M],
                    op=mybir.AluOpType.max,
                )
                s2 = scrp.tile([128, M // 4], F16, tag="s2")
                nc.vector.tensor_tensor(
                    s2[:], s1[:, 0 : M // 4], s1[:, M // 4 : M // 2],
                    op=mybir.AluOpType.max,
                )
                s3 = scrp.tile([128, M // 8], F16, tag="s3")
                nc.vector.tensor_tensor(
                    s3[:], s2[:, 0 : M // 8], s2[:, M // 8 : M // 4],
                    op=mybir.AluOpType.max,
                )
                nc.vector.tensor_reduce(
                    d01sb[:, i : i + 1],
                    s3[:],
                    axis=mybir.AxisListType.X,
                    op=mybir.AluOpType.max,
                )
                if acc_dve is None:
                    acc_dve = ctile
                else:
                    acc_new = acc0p.tile([128, M], F16, tag="accd")
                    nc.vector.tensor_tensor(
                        acc_new[:], acc_dve[:], ctile[:], op=mybir.AluOpType.max
                    )
                    acc_dve = acc_new
            acc = acc_dve

            # partition-axis max: PE-transpose acc 128x128 blocks into PSUM,
            # then one DVE X-reduce.
            d10t = outp.tile([128, M // 128], F32)
            psT = pp.tile([128, M], F16, tag="ps")
            for t in range(M // 128):
                nc.tensor.transpose(
                    psT[:, bass.ts(t, 128)], acc[:, bass.ts(t, 128)], id_sb[:]
                )
            nc.vector.tensor_reduce(
                d10t[:],
                psT[:].rearrange("p (t c) -> p t c", c=128),
                axis=mybir.AxisListType.X,
                op=mybir.AluOpType.max,
            )
            nc.sync.dma_start(negd10[:], d10t[:])
            nc.sync.dma_start(negd01[:], d01sb[:])
    _split_multi_waits(nc)
    return nc


_PROGRAM = None


def get_program() -> bass.Bass:
    global _PROGRAM
    if _PROGRAM is None:
        _PROGRAM = build_program()
    return _PROGRAM


def _split3(x: np.ndarray):
    bf = ml_dtypes.bfloat16
    h1 = x.astype(bf).astype(np.float32)
    h2 = (x - h1).astype(bf).astype(np.float32)
    h3 = (x - h1 - h2).astype(bf).astype(np.float32)
    return h1, h2, h3


# cross-product levels kept: everything with combined magnitude >= ~2^-27
_PAIRS = [(0, 0), (0, 1), (1, 0), (0, 2), (1, 1), (2, 0)]


def make_in_maps(template: np.ndarray, source: np.ndarray) -> list[dict]:
    """Host-side prep: split-bf16 augmented K=24 representations, sharded per
    core. Core c -> batch c//2, template half c%2."""
    template = np.asarray(template, dtype=np.float32)
    source = np.asarray(source, dtype=np.float32)
    bf = ml_dtypes.bfloat16
    in_maps = []
    for c in range(N_CORES):
        b, hh = divmod(c, 2)
        t = template[b, hh * HALF : (hh + 1) * HALF]  # [HALF, 3]
        s = source[b]  # [M, 3]
        T = _split3(t)
        U = _split3((2.0 * s).astype(np.float32))
        nt = (t.astype(np.float64) ** 2).sum(-1).astype(np.float32)
        ns = (s.astype(np.float64) ** 2).sum(-1).astype(np.float32)
        NT = _split3(nt)
        NS = _split3(ns)
        ones_t = np.ones_like(nt)
        ones_s = np.ones_like(ns)
        a_rows, b_rows = [], []
        for cc in range(3):
            for (ii, jj) in _PAIRS:
                a_rows.append(T[ii][:, cc])
                b_rows.append(U[jj][:, cc])
        for kk in range(3):
            a_rows.append(-NT[kk])
            b_rows.append(ones_s)
            a_rows.append(-ones_t)
            b_rows.append(NS[kk])
        tmplA = np.stack(a_rows, 0).astype(bf)  # [K, HALF]
        srcA = np.stack(b_rows, 0).astype(bf)  # [K, M]
        in_maps.append(
            {
                "tmplA": np.ascontiguousarray(tmplA),
                "srcA": np.ascontiguousarray(srcA),
                "ident": np.eye(128, dtype=np.float16),
            }
        )
    return in_maps


def combine(results: list[dict]) -> np.ndarray:
    """Gather per-core partials into the scalar loss (float64 accumulation)."""
    per_batch = []
    for b in range(B):
        r0, r1 = results[2 * b], results[2 * b + 1]
        d01_parts = []
        for r in (r0, r1):
            nd01 = r["negd01"].astype(np.float64)  # [128, 16]
            # template index within half = i*128 + p -> transpose to [16,128]
            d01_parts.append(nd01.T.reshape(-1))
        d01 = -np.concatenate(d01_parts)  # [4096]
        # negd10[n_loc, h*16+t] for source index h*2048 + t*128 + n_loc
        nd10 = np.maximum(
            r0["negd10"].astype(np.float64), r1["negd10"].astype(np.float64)
        )
        d10 = -nd10.T.reshape(-1)  # [32,128] -> index t'*128+n_loc with t'=h*16+t
        per_batch.append(d01.mean() + d10.mean())
    return np.asarray(np.mean(per_batch), dtype=np.float32)


def kernel(template: np.ndarray, source: np.ndarray) -> np.ndarray:
    nc = get_program()
    in_maps = make_in_maps(template, source)
    res = run_bass_kernel_spmd(nc, in_maps, list(range(N_CORES)))
    return combine(res.results)
